# revision 14
# baseline (speedup 1.0000x reference)
"""Trainium2 Bass kernel for nn_Attention_13348758356565.

Dense transformer attention block (B=16, N=1024 tokens, DIM=1024, 16 heads x 64)
with axial rotary embeddings, data-parallel over batch across 8 NeuronCores
(2 samples per core). All matmuls bf16 on TensorE at full rate.

Per sample:
- QKV projection x-stationary -> natural [tok, outdim] psum tiles [128, 512]
  (8 heads per half). Rotary applied by DVE directly from psum; V drains into
  [keys, 16*(64+1)] tiles with an interleaved ones column per head (free
  softmax denominator).
- Q/K/attn transposes are dma_start_transpose (xbar) calls: no PE transposes,
  no psum-drain copies. qT/kT layout: [dim%128, (t, dim//128, tok%128)].
- QK^T: kT-slice stationary [64, 128], qT moving [64, 4, 128] -> scores
  [keys, queries] psum; exp on ScalarE -> p bf16 in SBUF.
- P*V: p-chunk stationary [128 keys, 128 queries], moving V[keys, 65] ->
  [queries, 65] psum accumulated over key tiles at full PE rate; denominator
  is column 64 -> normalized by one broadcast tensor_tensor into natural
  attn layout (no partition broadcast needed).
- out-proj: attnT-stationary chunks vs wprojT moving; y written bf16 and
  upcast on host.

Cross-sample emission interleave keeps the PE stream dense during the
exp-gated attention phase so the HAM clock gate stays at 2.4 GHz.
"""

import os
import sys

sys.path.insert(0, "/opt/trn_rl_repo")

import dataclasses
import numpy as np

import concourse.bacc as bacc
import concourse.mybir as mybir
import concourse.tile as tile
from concourse import bass_utils

F32 = mybir.dt.float32
BF16 = mybir.dt.bfloat16
EXP = mybir.ActivationFunctionType.Exp

B, HF, WF = 16, 32, 32
DIM, NH, HD = 1024, 16, 64
N = HF * WF          # 1024 tokens
NCORES = 8
BPC = B // NCORES    # 2 samples per core
NT = N // 128        # 8 token tiles
ND = DIM // 128      # 8 contraction chunks
SCALE = 1.0 / np.sqrt(HD)

mul = mybir.AluOpType.mult
sub = mybir.AluOpType.subtract
add = mybir.AluOpType.add

last_exec_time_ns = None


def _bcast_mid(ap, count):
    """Insert a step-0 (broadcast) middle dim into a [P, C] AP -> [P, count, C]."""
    return dataclasses.replace(ap, ap=[ap.ap[0], [0, count], ap.ap[1]])


def _bcast_last(ap, count):
    """Append a step-0 (broadcast) last dim to an AP -> [..., count]."""
    return dataclasses.replace(ap, ap=list(ap.ap) + [[0, count]])


def _freq_tables():
    d = HD // 4
    base = (np.linspace(1.0, (HF * WF) / 2.0, d // 2, dtype=np.float64) * np.pi)
    posH = np.linspace(-1.0, 1.0, HF)
    posW = np.linspace(-1.0, 1.0, WF)
    fH = np.repeat(posH[:, None] * base[None, :], 2, axis=-1)   # [H, 16]
    fW = np.repeat(posW[:, None] * base[None, :], 2, axis=-1)   # [W, 16]
    fH = np.broadcast_to(fH[:, None, :], (HF, WF, d))
    fW = np.broadcast_to(fW[None, :, :], (HF, WF, d))
    freqs = np.concatenate([fH, fW], axis=-1).reshape(N, HD // 2)
    # freqs[:, 2i] == freqs[:, 2i+1]; keep one per pair -> [N, 16]
    half = freqs[:, 0::2].astype(np.float64)
    # [128, NT, 16]: row p, tile t -> token t*128+p
    cos = np.cos(half).astype(np.float32).reshape(NT, 128, 16).transpose(1, 0, 2)
    sin = np.sin(half).astype(np.float32).reshape(NT, 128, 16).transpose(1, 0, 2)
    return (np.ascontiguousarray(cos.reshape(128, NT * 16)),
            np.ascontiguousarray(sin.reshape(128, NT * 16)))


def _build():
    nc = bacc.Bacc("TRN2", target_bir_lowering=False, debug=False)

    xT_d = nc.dram_tensor("xT", [BPC, DIM, N], BF16, kind="ExternalInput")
    wqkvT_d = nc.dram_tensor("wqkvT", [DIM, 3 * DIM], BF16, kind="ExternalInput")
    wprojT_d = nc.dram_tensor("wprojT", [DIM, DIM], BF16, kind="ExternalInput")
    bproj_d = nc.dram_tensor("bproj", [1, DIM], BF16, kind="ExternalInput")
    ones_d = nc.dram_tensor("ones", [1, 128], BF16, kind="ExternalInput")
    cosn_d = nc.dram_tensor("cosn", [128, NT * 16], BF16, kind="ExternalInput")
    sinn_d = nc.dram_tensor("sinn", [128, NT * 16], BF16, kind="ExternalInput")
    y_d = nc.dram_tensor("y", [BPC, N, DIM], BF16, kind="ExternalOutput")

    with tile.TileContext(nc) as tc:
        with (
            tc.tile_pool(name="sb", bufs=1) as sb,
            tc.tile_pool(name="ps", bufs=1, space="PSUM") as ps,
        ):
            # ---------------- constants ----------------
            ones_r = sb.tile([1, 128], BF16, name="ones_r")
            nc.scalar.dma_start(ones_r[:], ones_d.ap())
            bproj = sb.tile([1, DIM], BF16, name="bproj")
            nc.scalar.dma_start(bproj[:], bproj_d.ap())
            cosn = sb.tile([128, NT * 16], BF16, name="cosn")
            sinn = sb.tile([128, NT * 16], BF16, name="sinn")
            nc.scalar.dma_start(cosn[:], cosn_d.ap())
            nc.scalar.dma_start(sinn[:], sinn_d.ap())
            wpr = sb.tile([128, ND, DIM], BF16, name="wpr")
            nc.scalar.dma_start(
                wpr[:], wprojT_d.ap().rearrange("(c p) o -> p c o", p=128))
            # warm the exp table set early (hides the ~2.7us table load)
            expwarm = sb.tile([1, 16], F32, name="expwarm")
            nc.scalar.activation(expwarm[:], cosn[0:1, 0:16], EXP, scale=1.0)

            # ---------------- per-sample inputs ----------------
            def xT_tiles(s):
                xt = sb.tile([128, ND, N], BF16, name=f"xT_s{s}", tag="xT")
                nc.scalar.dma_start(
                    xt[:],
                    xT_d.ap()[s].rearrange("(c p) n -> p c n", p=128))
                return xt

            wq_cache = {}

            def fetch_w(s, sect, half, eng=None):
                """stream the [DIM, 512] weight slab for (section, half) as one
                strided DMA: [128 part, 8 in-chunk, 512 outcols]."""
                wt = sb.tile([128, ND, 512], BF16,
                             name=f"wq_s{s}_{sect}_{half}", tag="wq", bufs=2)
                (eng or nc.sync).dma_start(
                    wt[:],
                    wqkvT_d.ap().rearrange("(c p) o -> p c o", p=128)
                    [:, :, sect * DIM + half * 512: sect * DIM + half * 512 + 512])
                wq_cache[(s, sect, half)] = wt

            def proj_psum(s, xT, sect, half, t, tag):
                """[128 tok, 512 outdims] psum tile (heads half*8..half*8+8)."""
                py = ps.tile([128, 512], F32, name=f"py_{tag}", tag="mm512", bufs=2)
                wt = wq_cache[(s, sect, half)]
                for d in range(ND):
                    nc.tensor.matmul(
                        py[:],
                        xT[:, d, t * 128:(t + 1) * 128],
                        wt[:, d, :],
                        start=(d == 0), stop=(d == ND - 1))
                return py

            def v_half(s, half, t, xT, vt):
                py = proj_psum(s, xT, 2, half, t, f"v{s}{half}{t}")
                vv = vt[:].rearrange("p (h c) -> p h c", c=HD + 1)
                h0 = half * 8
                nc.vector.memset(vv[:, h0:h0 + 8, HD], 1.0)
                nc.vector.tensor_copy(
                    vv[:, h0:h0 + 8, 0:HD],
                    py[:].rearrange("p (h c) -> p h c", c=HD))
                return vt

            def qk_half(s, sect, half, t, xT, dstT):
                """project half of q (sect=0) or k (sect=1) for token-tile t,
                rotary, transpose into dstT[:, t, half*4:(half+1)*4, :]."""
                py = proj_psum(s, xT, sect, half, t, f"s{sect}_{s}{half}{t}")
                pr = py[:].rearrange("p (h i u) -> p h i u", h=8, i=32, u=2)
                ev, od = pr[:, :, 0:16, 0], pr[:, :, 0:16, 1]
                cb = _bcast_mid(cosn[:, t * 16:(t + 1) * 16], 8)
                sbb = _bcast_mid(sinn[:, t * 16:(t + 1) * 16], 8)
                qn = sb.tile([128, 512], BF16, name=f"qn_{sect}_{s}{half}{t}",
                             tag="qn", bufs=2)
                qr = qn[:].rearrange("p (h i u) -> p h i u", h=8, i=32, u=2)
                t0 = sb.tile([128, 8, 16], BF16, name=f"t0_{sect}_{s}{half}{t}",
                             tag="rt0", bufs=1)
                t1 = sb.tile([128, 8, 16], BF16, name=f"t1_{sect}_{s}{half}{t}",
                             tag="rt1", bufs=1)
                nc.vector.tensor_tensor(t0[:], ev, cb, mul)
                nc.vector.tensor_tensor(t1[:], od, sbb, mul)
                nc.vector.tensor_tensor(qr[:, :, 0:16, 0], t0[:], t1[:], sub)
                t2 = sb.tile([128, 8, 16], BF16, name=f"t2_{sect}_{s}{half}{t}",
                             tag="rt0", bufs=1)
                t3 = sb.tile([128, 8, 16], BF16, name=f"t3_{sect}_{s}{half}{t}",
                             tag="rt1", bufs=1)
                nc.vector.tensor_tensor(t2[:], od, cb, mul)
                nc.vector.tensor_tensor(t3[:], ev, sbb, mul)
                nc.vector.tensor_tensor(qr[:, :, 0:16, 1], t2[:], t3[:], add)
                # pass-through dims 32:64 of each head
                pp = py[:].rearrange("p (h c) -> p h c", c=HD)
                qp = qn[:].rearrange("p (h c) -> p h c", c=HD)
                nc.vector.tensor_copy(qp[:, :, 32:64], pp[:, :, 32:64])
                # transpose [tok, dim-half] -> qT[:, t, half*4:(half+1)*4, :]
                nc.sync.dma_start_transpose(
                    dstT[:].rearrange("p (t c q) -> p t c q", t=NT, c=ND)
                    [:, t, half * 4:(half + 1) * 4, :],
                    qn[:])

            # ---------------- attention ----------------
            def attention_head(s, h, qTt, kTt, vts, at, nchs=(0, 1),
                               pacer=None):
                kv = kTt[:].rearrange("p (t c q) -> p t c q", t=NT, c=ND)
                qv = qTt[:].rearrange("p (t c q) -> p t c q", t=NT, c=ND)
                r0 = (h % 2) * 64
                for nch in nchs:
                    p_q = [sb.tile([128, 4, 512], BF16,
                                   name=f"p_s{s}h{h}n{nch}q{i}", tag="p", bufs=3)
                           for i in range(2)]
                    for mp in range(NT // 2):
                        st = ps.tile([128, 1024], F32, tag="st", bufs=2,
                                     name=f"st_{s}_{h}_{nch}_{mp}")
                        for u in range(2):
                            nc.tensor.matmul(
                                st[:, u * 512:(u + 1) * 512],
                                kv[r0:r0 + 64, mp * 2 + u, h // 2],
                                qv[r0:r0 + 64, nch * 4:(nch + 1) * 4, h // 2],
                            )
                        nc.scalar.activation(
                            p_q[mp // 2][:, (mp % 2) * 2:(mp % 2) * 2 + 2, :],
                            st[:], EXP, scale=float(SCALE))
                    if pacer is not None:
                        pacer.tick()
                    pv = ps.tile([128, 260], F32, name=f"pv_{s}_{h}_{nch}",
                                 tag="pv", bufs=2)
                    for ql in range(4):
                        for m in range(NT):
                            nc.tensor.matmul(
                                pv[:, ql * 65:ql * 65 + 65],
                                p_q[m // 4][:, m % 4, ql * 128:(ql + 1) * 128],
                                vts[m][:].rearrange("p (h c) -> p h c",
                                                    c=HD + 1)[:, h],
                                start=(m == 0), stop=(m == NT - 1))
                    pvv = pv[:].rearrange("p (q c) -> p q c", c=65)
                    rc = sb.tile([128, 4], F32, name=f"rc_{s}_{h}_{nch}",
                                 tag="rc", bufs=2)
                    nc.vector.reciprocal_approx_fast(rc[:], pvv[:, :, 64])
                    av = at[:].rearrange("p (q h c) -> p q h c", q=NT, h=NH)
                    nc.vector.tensor_tensor(
                        av[:, nch * 4:(nch + 1) * 4, h, :],
                        pvv[:, :, 0:64], _bcast_last(rc[:], HD), mul)
                    if pacer is not None:
                        pacer.tick()

            # ---------------- output projection ----------------
            def proj_out_half(s, qt, at, half, atq_box):
                if half == 0:
                    atq = sb.tile([128, ND, 128], BF16, name=f"atT_{s}_{qt}",
                                  tag="attnT", bufs=2)
                    nc.sync.dma_start_transpose(
                        atq[:], at[:, qt * 1024:(qt + 1) * 1024])
                    atq_box[qt] = atq
                atq = atq_box[qt]
                if True:
                    py = ps.tile([128, 512], F32, name=f"yp_{s}_{qt}_{half}",
                                 tag="mm512", bufs=2)
                    for d in range(ND):
                        nc.tensor.matmul(
                            py[:],
                            atq[:, d, :],
                            wpr[:, d, half * 512:(half + 1) * 512],
                            start=(d == 0), stop=False)
                    nc.tensor.matmul(
                        py[:],
                        ones_r[:], bproj[:, half * 512:(half + 1) * 512],
                        start=False, stop=True)
                    ysb = sb.tile([128, 512], BF16, name=f"y_{s}_{qt}_{half}",
                                  tag="ysb", bufs=2)
                    nc.vector.tensor_copy(ysb[:], py[:])
                    nc.sync.dma_start(
                        y_d.ap()[s, qt * 128:(qt + 1) * 128,
                                 half * 512:(half + 1) * 512],
                        ysb[:])

            atq_boxes = [{} for _ in range(BPC)]

            def proj_out(s, qt, at):
                for half in range(2):
                    proj_out_half(s, qt, at, half, atq_boxes[s])

            class Pacer:
                def __init__(self, items, total_slots):
                    self.items = items
                    self.total = max(total_slots, 1)
                    self.slot = 0
                    self.done = 0
                def _run(self, it):
                    if callable(it):
                        it()
                    else:
                        run_item(it)
                def tick(self):
                    self.slot += 1
                    want = min(len(self.items),
                               len(self.items) * self.slot // self.total)
                    while self.done < want:
                        self._run(self.items[self.done]); self.done += 1
                def drain(self):
                    while self.done < len(self.items):
                        self._run(self.items[self.done]); self.done += 1

            # ================= emission schedule =================
            qT = [sb.tile([128, NT * ND * 128], BF16, name=f"qT_s{s}", tag="qT",
                          bufs=2) for s in range(BPC)]
            kT = [sb.tile([128, NT * ND * 128], BF16, name=f"kT_s{s}", tag="kT",
                          bufs=2) for s in range(BPC)]
            attn = [sb.tile([128, NT * 1024], BF16, name=f"attn_s{s}", tag="attn",
                            bufs=2) for s in range(BPC)]
            vsb = [[sb.tile([128, NH * (HD + 1)], BF16, name=f"v_s{s}_{t}",
                            tag=f"v{t}", bufs=2) for t in range(NT)]
                   for s in range(BPC)]

            def slab_items(s, sect, half, xT):
                """fetch + the 8 per-t work items for one weight slab."""
                items = [("w", (s, sect, half))]
                for t in range(NT):
                    if sect == 2:
                        items.append(("v", (s, half, t, xT)))
                    else:
                        items.append(("qk", (s, sect, half, t, xT)))
                return items

            def run_item(it, weng=None):
                kind, args = it
                if kind == "w":
                    fetch_w(*args, eng=weng)
                elif kind == "v":
                    s_, half, t, xT = args
                    v_half(s_, half, t, xT, vsb[s_][t])
                else:
                    s_, sect, half, t, xT = args
                    qk_half(s_, sect, half, t, xT,
                            qT[s_] if sect == 0 else kT[s_])

            def run_slabs(slabs):
                """emit slab work with fetches hoisted 2 slabs ahead."""
                items = [slab_items(s_, sect, half, xT)
                         for (s_, sect, half, xT) in slabs]
                # reorder: fetch of slab i+2 goes before slab i's t-work
                out = []
                fetched = 0
                for i in range(len(items)):
                    while fetched <= min(i + 2, len(items) - 1):
                        out.append(items[fetched][0]); fetched += 1
                    out.extend(items[i][1:])
                return out

            # ---- phase P(s0): V + K/Q half0 of sample 0 (heads 0-7
            # of s0 only need these; half1 folds into A(s0)); the first
            # two attention heads interleave with the tail of Q-half0 ----
            xT0 = xT_tiles(0)
            p_items = run_slabs([(0, 2, 0, xT0), (0, 2, 1, xT0),
                                 (0, 1, 0, xT0), (0, 0, 0, xT0)])
            for it in p_items[:-4]:
                run_item(it, weng=nc.scalar)
            attention_head(0, 0, qT[0], kT[0], vsb[0], attn[0], nchs=(0,))
            for it in p_items[-4:]:
                run_item(it, weng=nc.scalar)
            attention_head(0, 0, qT[0], kT[0], vsb[0], attn[0], nchs=(1,))
            attention_head(0, 1, qT[0], kT[0], vsb[0], attn[0])

            # ---- phase A(s0): s0 attention; heads 2-7 absorb s0 K/Q half1
            # + start of s1 projection; heads 8-15 absorb the rest ----
            xT1 = sb.tile([128, ND, N], BF16, name="xT_s1", tag="xT")
            s1_filler = (run_slabs([(0, 1, 1, xT0), (0, 0, 1, xT0)])
                         + run_slabs([(1, 2, 0, xT1), (1, 2, 1, xT1),
                                      (1, 1, 0, xT1), (1, 0, 0, xT1)]))
            run_item(s1_filler[0])
            run_item(s1_filler[1])
            nc.sync.dma_start(
                xT1[:], xT_d.ap()[1].rearrange("(c p) n -> p c n", p=128))
            pac0 = Pacer(s1_filler[2:], (NH - 2) * 4)
            for h in range(2, NH):
                attention_head(0, h, qT[0], kT[0], vsb[0], attn[0], pacer=pac0)
            pac0.drain()

            # ---- phase A(s1): s1 attention (heads 0-7 ready) + s1 K/Q
            # half1 projection, then out-proj of s0 on the late heads ----
            late = run_slabs([(1, 1, 1, xT1), (1, 0, 1, xT1)])
            pjs = [(lambda qt=qt, half=half:
                    proj_out_half(0, qt, attn[0], half, atq_boxes[0]))
                   for qt in range(NT) for half in range(2)]
            pac1 = Pacer(late, 6 * 4)
            pac2 = Pacer(pjs, 10 * 4)
            for h in range(NH):
                attention_head(1, h, qT[1], kT[1], vsb[1], attn[1],
                               pacer=(pac1 if h < 6 else pac2))
            pac1.drain()
            pac2.drain()

            # ---- out-proj of s1 ----
            for qt in range(NT):
                proj_out(1, qt, attn[1])

    nc.compile()
    return nc


_NC_CACHE = None


def kernel(x, w_qkv, w_proj, b_proj):
    global _NC_CACHE, last_exec_time_ns
    import ml_dtypes

    x = np.asarray(x, np.float32)
    w_qkv = np.asarray(w_qkv, np.float32)
    w_proj = np.asarray(w_proj, np.float32)
    b_proj = np.asarray(b_proj, np.float32)

    if _NC_CACHE is None:
        _NC_CACHE = _build()
    nc = _NC_CACHE

    cosn, sinn = _freq_tables()
    cosn = cosn.astype(ml_dtypes.bfloat16)
    sinn = sinn.astype(ml_dtypes.bfloat16)
    wqkvT = np.ascontiguousarray(w_qkv.T).astype(ml_dtypes.bfloat16)
    wprojT = np.ascontiguousarray(w_proj.T).astype(ml_dtypes.bfloat16)
    bproj16 = b_proj.reshape(1, DIM).astype(ml_dtypes.bfloat16)
    ones16 = np.ones((1, 128), ml_dtypes.bfloat16)

    in_maps = []
    for c in range(NCORES):
        xs = x[c * BPC:(c + 1) * BPC]                       # [2, N, DIM]
        xT = np.ascontiguousarray(xs.transpose(0, 2, 1)).astype(ml_dtypes.bfloat16)
        in_maps.append({
            "xT": xT, "wqkvT": wqkvT, "wprojT": wprojT,
            "bproj": bproj16, "ones": ones16, "cosn": cosn, "sinn": sinn,
        })

    trace = bool(os.environ.get("KERNEL_TRACE"))
    kwargs = {}
    if trace:
        kwargs["trace"] = True
        td = os.environ.get("KERNEL_TRACE_DIR")
        if td:
            kwargs["tmpdir"] = td
    res = bass_utils.run_bass_kernel_spmd(
        nc, in_maps, core_ids=list(range(NCORES)), **kwargs)
    last_exec_time_ns = res.exec_time_ns
    out = np.concatenate([np.asarray(res.results[c]["y"]) for c in range(NCORES)],
                         axis=0)
    return np.ascontiguousarray(out.reshape(B, N, DIM).astype(np.float32))


if __name__ == "__main__":
    rng = np.random.default_rng(0)
    xs = rng.standard_normal((B, N, DIM), dtype=np.float32)
    wq = rng.standard_normal((3 * DIM, DIM), dtype=np.float32) / 32
    wp = rng.standard_normal((DIM, DIM), dtype=np.float32) / 32
    bp = np.zeros(DIM, np.float32)
    y = kernel(xs, wq, wp, bp)
    print("y", y.shape, y.dtype, float(np.abs(y).max()))


# revision 15
# speedup vs baseline: 1.1664x; 1.1664x over previous
"""Trainium2 Bass kernel for nn_Attention_13348758356565.

Dense transformer attention block (B=16, N=1024 tokens, DIM=1024, 16 heads x 64)
with axial rotary embeddings, data-parallel over batch across 8 NeuronCores
(2 samples per core). All matmuls bf16 on TensorE at full rate.

Per sample:
- QKV projection x-stationary -> natural [tok, outdim] psum tiles [128, 512]
  (8 heads per half). Rotary applied by DVE directly from psum; V drains into
  [keys, 16*(64+1)] tiles with an interleaved ones column per head (free
  softmax denominator).
- Q/K/attn transposes are dma_start_transpose (xbar) calls: no PE transposes,
  no psum-drain copies. qT/kT layout: [dim%128, (t, dim//128, tok%128)].
- QK^T: kT-slice stationary [64, 128], qT moving [64, 4, 128] -> scores
  [keys, queries] psum; exp on ScalarE -> p bf16 in SBUF.
- P*V: p-chunk stationary [128 keys, 128 queries], moving V[keys, 65] ->
  [queries, 65] psum accumulated over key tiles at full PE rate; denominator
  is column 64 -> normalized by one broadcast tensor_tensor into natural
  attn layout (no partition broadcast needed).
- out-proj: attnT-stationary chunks vs wprojT moving; y written bf16 and
  upcast on host.

Cross-sample emission interleave keeps the PE stream dense during the
exp-gated attention phase so the HAM clock gate stays at 2.4 GHz.
"""

import os
import sys

sys.path.insert(0, "/opt/trn_rl_repo")

import dataclasses
import numpy as np

import concourse.bacc as bacc
import concourse.mybir as mybir
import concourse.tile as tile
from concourse import bass_utils

F32 = mybir.dt.float32
BF16 = mybir.dt.bfloat16
EXP = mybir.ActivationFunctionType.Exp

B, HF, WF = 16, 32, 32
DIM, NH, HD = 1024, 16, 64
N = HF * WF          # 1024 tokens
NCORES = 8
BPC = B // NCORES    # 2 samples per core
NT = N // 128        # 8 token tiles
ND = DIM // 128      # 8 contraction chunks
SCALE = 1.0 / np.sqrt(HD)

mul = mybir.AluOpType.mult
sub = mybir.AluOpType.subtract
add = mybir.AluOpType.add

last_exec_time_ns = None


def _bcast_mid(ap, count):
    """Insert a step-0 (broadcast) middle dim into a [P, C] AP -> [P, count, C]."""
    return dataclasses.replace(ap, ap=[ap.ap[0], [0, count], ap.ap[1]])


def _bcast_last(ap, count):
    """Append a step-0 (broadcast) last dim to an AP -> [..., count]."""
    return dataclasses.replace(ap, ap=list(ap.ap) + [[0, count]])


def _freq_tables():
    d = HD // 4
    base = (np.linspace(1.0, (HF * WF) / 2.0, d // 2, dtype=np.float64) * np.pi)
    posH = np.linspace(-1.0, 1.0, HF)
    posW = np.linspace(-1.0, 1.0, WF)
    fH = np.repeat(posH[:, None] * base[None, :], 2, axis=-1)   # [H, 16]
    fW = np.repeat(posW[:, None] * base[None, :], 2, axis=-1)   # [W, 16]
    fH = np.broadcast_to(fH[:, None, :], (HF, WF, d))
    fW = np.broadcast_to(fW[None, :, :], (HF, WF, d))
    freqs = np.concatenate([fH, fW], axis=-1).reshape(N, HD // 2)
    # freqs[:, 2i] == freqs[:, 2i+1]; keep one per pair -> [N, 16]
    half = freqs[:, 0::2].astype(np.float64)
    # [128, NT, 16]: row p, tile t -> token t*128+p
    cos = np.cos(half).astype(np.float32).reshape(NT, 128, 16).transpose(1, 0, 2)
    sin = np.sin(half).astype(np.float32).reshape(NT, 128, 16).transpose(1, 0, 2)
    return (np.ascontiguousarray(cos.reshape(128, NT * 16)),
            np.ascontiguousarray(sin.reshape(128, NT * 16)))


def _build():
    nc = bacc.Bacc("TRN2", target_bir_lowering=False, debug=False)

    xT_d = nc.dram_tensor("xT", [BPC, DIM, N], BF16, kind="ExternalInput")
    wqkvT_d = nc.dram_tensor("wqkvT", [DIM, 3 * DIM], BF16, kind="ExternalInput")
    wprojT_d = nc.dram_tensor("wprojT", [DIM, DIM], BF16, kind="ExternalInput")
    bproj_d = nc.dram_tensor("bproj", [1, DIM], BF16, kind="ExternalInput")
    ones_d = nc.dram_tensor("ones", [1, 128], BF16, kind="ExternalInput")
    cosn_d = nc.dram_tensor("cosn", [128, NT * 16], BF16, kind="ExternalInput")
    sinn_d = nc.dram_tensor("sinn", [128, NT * 16], BF16, kind="ExternalInput")
    y_d = nc.dram_tensor("y", [BPC, N, DIM], BF16, kind="ExternalOutput")

    with tile.TileContext(nc) as tc:
        with (
            tc.tile_pool(name="sb", bufs=1) as sb,
            tc.tile_pool(name="ps", bufs=1, space="PSUM") as ps,
        ):
            # ---------------- constants ----------------
            ones_r = sb.tile([1, 128], BF16, name="ones_r")
            nc.scalar.dma_start(ones_r[:], ones_d.ap())
            bproj = sb.tile([1, DIM], BF16, name="bproj")
            nc.scalar.dma_start(bproj[:], bproj_d.ap())
            cosn = sb.tile([128, NT * 16], BF16, name="cosn")
            sinn = sb.tile([128, NT * 16], BF16, name="sinn")
            nc.scalar.dma_start(cosn[:], cosn_d.ap())
            nc.scalar.dma_start(sinn[:], sinn_d.ap())
            wpr = sb.tile([128, ND, DIM], BF16, name="wpr")
            nc.scalar.dma_start(
                wpr[:], wprojT_d.ap().rearrange("(c p) o -> p c o", p=128))
            # warm the exp table set early (hides the ~2.7us table load)
            expwarm = sb.tile([1, 16], F32, name="expwarm")
            nc.scalar.activation(expwarm[:], cosn[0:1, 0:16], EXP, scale=1.0)

            # ---------------- per-sample inputs ----------------
            def xT_tiles(s):
                xt = sb.tile([128, ND, N], BF16, name=f"xT_s{s}", tag="xT")
                nc.scalar.dma_start(
                    xt[:],
                    xT_d.ap()[s].rearrange("(c p) n -> p c n", p=128))
                return xt

            wq_cache = {}

            def fetch_w(s, sect, half, eng=None):
                """stream the [DIM, 512] weight slab for (section, half) as one
                strided DMA: [128 part, 8 in-chunk, 512 outcols]."""
                wt = sb.tile([128, ND, 512], BF16,
                             name=f"wq_s{s}_{sect}_{half}", tag="wq", bufs=2)
                (eng or nc.sync).dma_start(
                    wt[:],
                    wqkvT_d.ap().rearrange("(c p) o -> p c o", p=128)
                    [:, :, sect * DIM + half * 512: sect * DIM + half * 512 + 512])
                wq_cache[(s, sect, half)] = wt

            def proj_psum(s, xT, sect, half, t, tag):
                """[128 tok, 512 outdims] psum tile (heads half*8..half*8+8)."""
                py = ps.tile([128, 512], F32, name=f"py_{tag}", tag="mm512", bufs=2)
                wt = wq_cache[(s, sect, half)]
                for d in range(ND):
                    nc.tensor.matmul(
                        py[:],
                        xT[:, d, t * 128:(t + 1) * 128],
                        wt[:, d, :],
                        start=(d == 0), stop=(d == ND - 1))
                return py

            def v_half(s, half, t, xT, vt, ceng=None):
                py = proj_psum(s, xT, 2, half, t, f"v{s}{half}{t}")
                vv = vt[:].rearrange("p (h c) -> p h c", c=HD + 1)
                h0 = half * 8
                nc.vector.memset(vv[:, h0:h0 + 8, HD], 1.0)
                if ceng is None:
                    nc.vector.tensor_copy(
                        vv[:, h0:h0 + 8, 0:HD],
                        py[:].rearrange("p (h c) -> p h c", c=HD))
                else:
                    ceng.copy(vv[:, h0:h0 + 8, 0:HD],
                              py[:].rearrange("p (h c) -> p h c", c=HD))
                return vt

            def qk_half(s, sect, half, t, xT, dstT, ceng=None):
                """project half of q (sect=0) or k (sect=1) for token-tile t,
                rotary, transpose into dstT[:, t, half*4:(half+1)*4, :]."""
                py = proj_psum(s, xT, sect, half, t, f"s{sect}_{s}{half}{t}")
                pr = py[:].rearrange("p (h i u) -> p h i u", h=8, i=32, u=2)
                ev, od = pr[:, :, 0:16, 0], pr[:, :, 0:16, 1]
                cb = _bcast_mid(cosn[:, t * 16:(t + 1) * 16], 8)
                sbb = _bcast_mid(sinn[:, t * 16:(t + 1) * 16], 8)
                qn = sb.tile([128, 512], BF16, name=f"qn_{sect}_{s}{half}{t}",
                             tag="qn", bufs=2)
                qr = qn[:].rearrange("p (h i u) -> p h i u", h=8, i=32, u=2)
                t0 = sb.tile([128, 8, 16], BF16, name=f"t0_{sect}_{s}{half}{t}",
                             tag="rt0", bufs=1)
                t1 = sb.tile([128, 8, 16], BF16, name=f"t1_{sect}_{s}{half}{t}",
                             tag="rt1", bufs=1)
                nc.vector.tensor_tensor(t0[:], ev, cb, mul)
                nc.vector.tensor_tensor(t1[:], od, sbb, mul)
                nc.vector.tensor_tensor(qr[:, :, 0:16, 0], t0[:], t1[:], sub)
                t2 = sb.tile([128, 8, 16], BF16, name=f"t2_{sect}_{s}{half}{t}",
                             tag="rt0", bufs=1)
                t3 = sb.tile([128, 8, 16], BF16, name=f"t3_{sect}_{s}{half}{t}",
                             tag="rt1", bufs=1)
                nc.vector.tensor_tensor(t2[:], od, cb, mul)
                nc.vector.tensor_tensor(t3[:], ev, sbb, mul)
                nc.vector.tensor_tensor(qr[:, :, 0:16, 1], t2[:], t3[:], add)
                # pass-through dims 32:64 of each head
                pp = py[:].rearrange("p (h c) -> p h c", c=HD)
                qp = qn[:].rearrange("p (h c) -> p h c", c=HD)
                if ceng is None:
                    nc.vector.tensor_copy(qp[:, :, 32:64], pp[:, :, 32:64])
                else:
                    ceng.copy(qp[:, :, 32:64], pp[:, :, 32:64])
                # transpose [tok, dim-half] -> qT[:, t, half*4:(half+1)*4, :]
                nc.sync.dma_start_transpose(
                    dstT[:].rearrange("p (t c q) -> p t c q", t=NT, c=ND)
                    [:, t, half * 4:(half + 1) * 4, :],
                    qn[:])

            # ---------------- attention ----------------
            def attention_head(s, h, qTt, kTt, vts, at, nchs=(0, 1),
                               pacer=None):
                kv = kTt[:].rearrange("p (t c q) -> p t c q", t=NT, c=ND)
                qv = qTt[:].rearrange("p (t c q) -> p t c q", t=NT, c=ND)
                r0 = (h % 2) * 64
                for nch in nchs:
                    p_q = [sb.tile([128, 4, 512], BF16,
                                   name=f"p_s{s}h{h}n{nch}q{i}", tag="p", bufs=3)
                           for i in range(2)]
                    for mp in range(NT // 2):
                        st = ps.tile([128, 1024], F32, tag="st", bufs=2,
                                     name=f"st_{s}_{h}_{nch}_{mp}")
                        for u in range(2):
                            nc.tensor.matmul(
                                st[:, u * 512:(u + 1) * 512],
                                kv[r0:r0 + 64, mp * 2 + u, h // 2],
                                qv[r0:r0 + 64, nch * 4:(nch + 1) * 4, h // 2],
                            )
                        nc.scalar.activation(
                            p_q[mp // 2][:, (mp % 2) * 2:(mp % 2) * 2 + 2, :],
                            st[:], EXP, scale=float(SCALE))
                    pv = ps.tile([128, 260], F32, name=f"pv_{s}_{h}_{nch}",
                                 tag="pv", bufs=2)
                    for ql in range(4):
                        for m in range(NT):
                            nc.tensor.matmul(
                                pv[:, ql * 65:ql * 65 + 65],
                                p_q[m // 4][:, m % 4, ql * 128:(ql + 1) * 128],
                                vts[m][:].rearrange("p (h c) -> p h c",
                                                    c=HD + 1)[:, h],
                                start=(m == 0), stop=(m == NT - 1))
                    pvv = pv[:].rearrange("p (q c) -> p q c", c=65)
                    rc = sb.tile([128, 4], F32, name=f"rc_{s}_{h}_{nch}",
                                 tag="rc", bufs=2)
                    nc.vector.reciprocal_approx_fast(rc[:], pvv[:, :, 64])
                    av = at[:].rearrange("p (q h c) -> p q h c", q=NT, h=NH)
                    nc.vector.tensor_tensor(
                        av[:, nch * 4:(nch + 1) * 4, h, :],
                        pvv[:, :, 0:64], _bcast_last(rc[:], HD), mul)
                    if pacer is not None:
                        pacer.tick()

            # ---------------- output projection ----------------
            def proj_out_half(s, qt, at, half, atq_box):
                if half == 0:
                    atq = sb.tile([128, ND, 128], BF16, name=f"atT_{s}_{qt}",
                                  tag="attnT", bufs=2)
                    nc.sync.dma_start_transpose(
                        atq[:], at[:, qt * 1024:(qt + 1) * 1024])
                    atq_box[qt] = atq
                atq = atq_box[qt]
                if True:
                    py = ps.tile([128, 512], F32, name=f"yp_{s}_{qt}_{half}",
                                 tag="mm512", bufs=2)
                    for d in range(ND):
                        nc.tensor.matmul(
                            py[:],
                            atq[:, d, :],
                            wpr[:, d, half * 512:(half + 1) * 512],
                            start=(d == 0), stop=False)
                    nc.tensor.matmul(
                        py[:],
                        ones_r[:], bproj[:, half * 512:(half + 1) * 512],
                        start=False, stop=True)
                    ysb = sb.tile([128, 512], BF16, name=f"y_{s}_{qt}_{half}",
                                  tag="ysb", bufs=2)
                    nc.vector.tensor_copy(ysb[:], py[:])
                    nc.sync.dma_start(
                        y_d.ap()[s, qt * 128:(qt + 1) * 128,
                                 half * 512:(half + 1) * 512],
                        ysb[:])

            atq_boxes = [{} for _ in range(BPC)]

            def proj_out(s, qt, at):
                for half in range(2):
                    proj_out_half(s, qt, at, half, atq_boxes[s])

            class Pacer:
                def __init__(self, items, total_slots):
                    self.items = items
                    self.total = max(total_slots, 1)
                    self.slot = 0
                    self.done = 0
                def _run(self, it):
                    if callable(it):
                        it()
                    else:
                        run_item(it)
                def tick(self):
                    self.slot += 1
                    want = min(len(self.items),
                               len(self.items) * self.slot // self.total)
                    while self.done < want:
                        self._run(self.items[self.done]); self.done += 1
                def drain(self):
                    while self.done < len(self.items):
                        self._run(self.items[self.done]); self.done += 1

            # ================= emission schedule =================
            qT = [sb.tile([128, NT * ND * 128], BF16, name=f"qT_s{s}", tag="qT",
                          bufs=2) for s in range(BPC)]
            kT = [sb.tile([128, NT * ND * 128], BF16, name=f"kT_s{s}", tag="kT",
                          bufs=2) for s in range(BPC)]
            attn = [sb.tile([128, NT * 1024], BF16, name=f"attn_s{s}", tag="attn",
                            bufs=2) for s in range(BPC)]
            vsb = [[sb.tile([128, NH * (HD + 1)], BF16, name=f"v_s{s}_{t}",
                            tag=f"v{t}", bufs=2) for t in range(NT)]
                   for s in range(BPC)]

            def slab_items(s, sect, half, xT):
                """fetch + the 8 per-t work items for one weight slab."""
                items = [("w", (s, sect, half))]
                for t in range(NT):
                    if sect == 2:
                        items.append(("v", (s, half, t, xT)))
                    else:
                        items.append(("qk", (s, sect, half, t, xT)))
                return items

            def run_item(it, weng=None, ceng=None):
                kind, args = it
                if kind == "w":
                    fetch_w(*args, eng=weng)
                elif kind == "v":
                    s_, half, t, xT = args
                    v_half(s_, half, t, xT, vsb[s_][t], ceng=ceng)
                else:
                    s_, sect, half, t, xT = args
                    qk_half(s_, sect, half, t, xT,
                            qT[s_] if sect == 0 else kT[s_], ceng=ceng)

            def run_slabs(slabs):
                """emit slab work with fetches hoisted 2 slabs ahead."""
                items = [slab_items(s_, sect, half, xT)
                         for (s_, sect, half, xT) in slabs]
                # reorder: fetch of slab i+2 goes before slab i's t-work
                out = []
                fetched = 0
                for i in range(len(items)):
                    while fetched <= min(i + 2, len(items) - 1):
                        out.append(items[fetched][0]); fetched += 1
                    out.extend(items[i][1:])
                return out

            # ---- phase P(s0): V + K/Q half0 of sample 0 (heads 0-7
            # of s0 only need these; half1 folds into A(s0)); the first
            # two attention heads interleave with the tail of Q-half0 ----
            xT0 = xT_tiles(0)
            p_items = run_slabs([(0, 2, 0, xT0), (0, 2, 1, xT0),
                                 (0, 1, 0, xT0), (0, 0, 0, xT0)])
            for it in p_items[:-4]:
                run_item(it, weng=nc.scalar, ceng=nc.scalar)
            attention_head(0, 0, qT[0], kT[0], vsb[0], attn[0], nchs=(0,))
            for it in p_items[-4:]:
                run_item(it, weng=nc.scalar, ceng=nc.scalar)
            attention_head(0, 0, qT[0], kT[0], vsb[0], attn[0], nchs=(1,))
            attention_head(0, 1, qT[0], kT[0], vsb[0], attn[0])

            # ---- phase A(s0): s0 attention; heads 2-7 absorb s0 K/Q half1
            # + start of s1 projection; heads 8-15 absorb the rest ----
            xT1 = sb.tile([128, ND, N], BF16, name="xT_s1", tag="xT")
            s1_filler = (run_slabs([(0, 1, 1, xT0), (0, 0, 1, xT0)])
                         + run_slabs([(1, 2, 0, xT1), (1, 2, 1, xT1),
                                      (1, 1, 0, xT1), (1, 0, 0, xT1)]))
            run_item(s1_filler[0])
            run_item(s1_filler[1])
            nc.sync.dma_start(
                xT1[:], xT_d.ap()[1].rearrange("(c p) n -> p c n", p=128))
            pac0 = Pacer(s1_filler[2:], (NH - 2) * 2)
            for h in range(2, NH):
                attention_head(0, h, qT[0], kT[0], vsb[0], attn[0], pacer=pac0)
            pac0.drain()

            # ---- phase A(s1): s1 attention (heads 0-7 ready) + s1 K/Q
            # half1 projection, then out-proj of s0 on the late heads ----
            late = run_slabs([(1, 1, 1, xT1), (1, 0, 1, xT1)])
            pjs = [(lambda qt=qt, half=half:
                    proj_out_half(0, qt, attn[0], half, atq_boxes[0]))
                   for qt in range(NT) for half in range(2)]
            pac1 = Pacer(late, 6 * 2)
            pac2 = Pacer(pjs, 10 * 2)
            for h in range(NH):
                attention_head(1, h, qT[1], kT[1], vsb[1], attn[1],
                               pacer=(pac1 if h < 6 else pac2))
            pac1.drain()
            pac2.drain()

            # ---- out-proj of s1 ----
            for qt in range(NT):
                proj_out(1, qt, attn[1])

    nc.compile()
    return nc


_NC_CACHE = None


def kernel(x, w_qkv, w_proj, b_proj):
    global _NC_CACHE, last_exec_time_ns
    import ml_dtypes

    x = np.asarray(x, np.float32)
    w_qkv = np.asarray(w_qkv, np.float32)
    w_proj = np.asarray(w_proj, np.float32)
    b_proj = np.asarray(b_proj, np.float32)

    if _NC_CACHE is None:
        _NC_CACHE = _build()
    nc = _NC_CACHE

    cosn, sinn = _freq_tables()
    cosn = cosn.astype(ml_dtypes.bfloat16)
    sinn = sinn.astype(ml_dtypes.bfloat16)
    wqkvT = np.ascontiguousarray(w_qkv.T).astype(ml_dtypes.bfloat16)
    wprojT = np.ascontiguousarray(w_proj.T).astype(ml_dtypes.bfloat16)
    bproj16 = b_proj.reshape(1, DIM).astype(ml_dtypes.bfloat16)
    ones16 = np.ones((1, 128), ml_dtypes.bfloat16)

    in_maps = []
    for c in range(NCORES):
        xs = x[c * BPC:(c + 1) * BPC]                       # [2, N, DIM]
        xT = np.ascontiguousarray(xs.transpose(0, 2, 1)).astype(ml_dtypes.bfloat16)
        in_maps.append({
            "xT": xT, "wqkvT": wqkvT, "wprojT": wprojT,
            "bproj": bproj16, "ones": ones16, "cosn": cosn, "sinn": sinn,
        })

    trace = bool(os.environ.get("KERNEL_TRACE"))
    kwargs = {}
    if trace:
        kwargs["trace"] = True
        td = os.environ.get("KERNEL_TRACE_DIR")
        if td:
            kwargs["tmpdir"] = td
    res = bass_utils.run_bass_kernel_spmd(
        nc, in_maps, core_ids=list(range(NCORES)), **kwargs)
    last_exec_time_ns = res.exec_time_ns
    out = np.concatenate([np.asarray(res.results[c]["y"]) for c in range(NCORES)],
                         axis=0)
    return np.ascontiguousarray(out.reshape(B, N, DIM).astype(np.float32))


if __name__ == "__main__":
    rng = np.random.default_rng(0)
    xs = rng.standard_normal((B, N, DIM), dtype=np.float32)
    wq = rng.standard_normal((3 * DIM, DIM), dtype=np.float32) / 32
    wp = rng.standard_normal((DIM, DIM), dtype=np.float32) / 32
    bp = np.zeros(DIM, np.float32)
    y = kernel(xs, wq, wp, bp)
    print("y", y.shape, y.dtype, float(np.abs(y).max()))
